# revision 22
# baseline (speedup 1.0000x reference)
"""Causal single-head attention (N=4096, D=F=1024) on 8 TRN2 NeuronCores.

Causally load-balanced sequence sharding: core c owns the four 128-row query
blocks {31-c, 23-c, 15-c, 7-c}. Keys/values are rotated by c tiles (junk
zeros in tiles t<c) so each core runs one uniform SPMD program in which key
tile t is matmul'd against a compile-time prefix of the query columns
(512/384/256/128 wide for t in [0,8)/[8,16)/[16,24)/[24,32)) - the
shrinking prefix implements the causal structure at tile granularity and
cuts score/AV matmul work to 62.5% of the full rectangle. Diagonal blocks
(t = 7,15,23,31) get an on-chip triangular affine_select on their last 128
columns. Softmax normalization + output bias are applied host-side on the
unnormalized projected output (linear, so exact).

Two SPMD launches:
  A) QKV projection for the core's own contiguous 512 rows.
  B) scores+exp / rowsum / AV / output projection on the shuffled blocks.
Matmul operands bf16 (f32 PSUM accumulation); host pre-blocks all tensors
so every DMA is ~128 descriptors of >=2KB contiguous per partition.
"""

import sys

try:
    import concourse.bass as bass
except ImportError:  # pragma: no cover
    sys.path.insert(0, "/opt/trn_rl_repo")
    import concourse.bass as bass

import ml_dtypes
import numpy as np

import concourse.mybir as mybir
import concourse.tile as tile
from concourse import bacc
from concourse.bass_utils import run_bass_kernel_spmd

N, D, F = 4096, 1024, 1024
C = 8              # cores
NL = N // C        # 512 query rows per core
P = 128
SCALE = 1.0 / float(np.sqrt(np.float32(F)))

F32 = mybir.dt.float32
MM_DT = mybir.dt.bfloat16  # matmul operand dtype (PSUM accumulation stays f32)

DT = D // P        # 8 contraction tiles
FT = F // P        # 8 f tiles
MT = N // P        # 32 key tiles
NT2 = NL // P      # 4 query-row tiles per core

WARMUP_A = 10
WARMUP_B = 10

# column width of key tile t (prefix of the query columns)
def _lw(t):
    return 512 - 128 * (t // 8)


# Filled with [launchA_ns, launchB_ns] when BASS_TRACE=1 profiling is active.
LAST_EXEC_NS = [None, None]
LAST_RESULTS = [None, None]

_CACHE = {}


def _build_qkv():
    nc = bacc.Bacc(None, target_bir_lowering=False)
    xT = nc.dram_tensor("xT", [P, DT, NL], MM_DT, kind="ExternalInput")
    wqb = nc.dram_tensor("wqb", [FT, P, DT, P], MM_DT, kind="ExternalInput")
    wkb = nc.dram_tensor("wkb", [FT, P, DT, P], MM_DT, kind="ExternalInput")
    wvb = nc.dram_tensor("wvb", [2, P, DT, 512], MM_DT, kind="ExternalInput")
    bq = nc.dram_tensor("bq", [P, FT], F32, kind="ExternalInput")
    bk = nc.dram_tensor("bk", [P, FT], F32, kind="ExternalInput")
    bvB = nc.dram_tensor("bvB", [P, F], F32, kind="ExternalInput")
    qT_o = nc.dram_tensor("qT_o", [F, NL], MM_DT, kind="ExternalOutput")
    kT_o = nc.dram_tensor("kT_o", [F, NL], MM_DT, kind="ExternalOutput")
    v_o = nc.dram_tensor("v_o", [NL, F], MM_DT, kind="ExternalOutput")

    with tile.TileContext(nc) as tc:
        with (
            tc.tile_pool(name="singles", bufs=1) as singles,
            tc.tile_pool(name="weights", bufs=8) as weights,
            tc.tile_pool(name="osb", bufs=6) as opool,
            tc.tile_pool(name="psum", bufs=6, space="PSUM") as psum,
        ):
            warm = singles.tile([P, NL], MM_DT)
            nc.vector.memset(warm, 0.0)
            wps = psum.tile([P, NL], F32, tag="ps")
            for wi in range(WARMUP_A):
                nc.tensor.matmul(
                    wps,
                    warm[:, :P],
                    warm,
                    start=(wi == 0),
                    stop=(wi == WARMUP_A - 1),
                )
            # first two weight chunks prefetched ahead so MM0 starts early;
            # xT quarters fan out across all four DMA-capable queues
            wc0 = weights.tile([P, DT, P], MM_DT, tag="wc")
            nc.sync.dma_start(out=wc0, in_=wqb.ap()[0])
            wc1 = weights.tile([P, DT, P], MM_DT, tag="wc")
            nc.scalar.dma_start(out=wc1, in_=wqb.ap()[1])
            xT_sb = singles.tile([P, DT, NL], MM_DT)
            for qi, eng in (
                (0, nc.sync),
                (1, nc.scalar),
                (2, nc.gpsimd),
                (3, nc.gpsimd),
            ):
                sl = slice(qi * (DT // 4), (qi + 1) * (DT // 4))
                eng.dma_start(out=xT_sb[:, sl, :], in_=xT.ap()[:, sl, :])
            bq_sb = singles.tile([P, FT], F32)
            nc.gpsimd.dma_start(out=bq_sb, in_=bq.ap())
            bk_sb = singles.tile([P, FT], F32)
            nc.gpsimd.dma_start(out=bk_sb, in_=bk.ap())
            bvB_sb = singles.tile([P, F], F32)
            nc.gpsimd.dma_start(out=bvB_sb, in_=bvB.ap())

            # q.T / k.T : out[f_tile, n] = sum_d wT[d, f] * xT[d, n]
            for wi, (w_t, b_sb, out_t) in enumerate(
                ((wqb, bq_sb, qT_o), (wkb, bk_sb, kT_o))
            ):
                for ft in range(FT):
                    idx = wi * FT + ft
                    if idx == 0:
                        wc = wc0
                    elif idx == 1:
                        wc = wc1
                    else:
                        wc = weights.tile([P, DT, P], MM_DT, tag="wc")
                        weng = nc.sync if idx % 2 == 0 else nc.scalar
                        weng.dma_start(out=wc, in_=w_t.ap()[ft])
                    ps = psum.tile([P, NL], F32, tag="ps")
                    for dt_i in range(DT):
                        nc.tensor.matmul(
                            ps,
                            wc[:, dt_i, :],
                            xT_sb[:, dt_i, :],
                            start=(dt_i == 0),
                            stop=(dt_i == DT - 1),
                        )
                    osb = opool.tile([P, NL], MM_DT, tag="osb")
                    nc.vector.tensor_scalar_add(
                        out=osb, in0=ps, scalar1=b_sb[:, ft : ft + 1]
                    )
                    oeng = nc.scalar if idx % 2 == 0 else nc.sync
                    oeng.dma_start(
                        out=out_t.ap()[ft * P : (ft + 1) * P, :], in_=osb
                    )

            # v : out[m_tile, f] = sum_d xT[d, m] * wvT[d, f]
            for fc in range(2):
                fs = slice(fc * 512, (fc + 1) * 512)
                wvc = weights.tile([P, DT, 512], MM_DT, tag="wvc")
                nc.gpsimd.dma_start(out=wvc, in_=wvb.ap()[fc])
                for mi in range(NT2):
                    ps = psum.tile([P, 512], F32, tag="ps")
                    for dt_i in range(DT):
                        nc.tensor.matmul(
                            ps,
                            xT_sb[:, dt_i, mi * P : (mi + 1) * P],
                            wvc[:, dt_i, :],
                            start=(dt_i == 0),
                            stop=(dt_i == DT - 1),
                        )
                    vsb = opool.tile([P, 512], MM_DT, tag="osb")
                    nc.vector.tensor_add(out=vsb, in0=ps, in1=bvB_sb[:, fs])
                    nc.gpsimd.dma_start(
                        out=v_o.ap()[mi * P : (mi + 1) * P, fs], in_=vsb
                    )
    nc.finalize()
    return nc


def _build_attn():
    nc = bacc.Bacc(None, target_bir_lowering=False)
    qT = nc.dram_tensor("qT", [P, FT, NL], MM_DT, kind="ExternalInput")
    kbs = nc.dram_tensor("kbs", [MT, P, FT, P], MM_DT, kind="ExternalInput")
    vbk = nc.dram_tensor("vbk", [FT, 2, P, MT // 2, P], MM_DT, kind="ExternalInput")
    ones = nc.dram_tensor("ones", [P, MT], MM_DT, kind="ExternalInput")
    projT = nc.dram_tensor("projT", [F, F], MM_DT, kind="ExternalInput")
    out_o = nc.dram_tensor("out_o", [NL, F], F32, kind="ExternalOutput")
    rs_o = nc.dram_tensor("rs_o", [1, NL], F32, kind="ExternalOutput")

    with tile.TileContext(nc) as tc:
        with (
            tc.tile_pool(name="singles", bufs=1) as singles,
            tc.tile_pool(name="kc", bufs=10) as kpool,
            tc.tile_pool(name="vc", bufs=6) as vpool,
            tc.tile_pool(name="osb", bufs=3) as opool,
            tc.tile_pool(name="sps", bufs=3, space="PSUM") as spsum,
            tc.tile_pool(name="rps", bufs=1, space="PSUM") as rpsum,
            tc.tile_pool(name="zps", bufs=2, space="PSUM") as zpsum,
            tc.tile_pool(name="ops", bufs=2, space="PSUM") as opsum,
        ):
            warm = singles.tile([P, NL], MM_DT)
            nc.vector.memset(warm, 0.0)
            wps = spsum.tile([P, NL], F32, tag="sps")
            for wi in range(WARMUP_B):
                nc.tensor.matmul(
                    wps,
                    warm[:, :P],
                    warm,
                    start=(wi == 0),
                    stop=(wi == WARMUP_B - 1),
                )
            # interleaved ramp: qT chunks + first 8 key tiles spread over all
            # three DMA queues so the ~1.5MB critical set lands fastest.
            LOOKAHEAD = 8
            kcs = {}

            def _kc_dma(t, eng):
                kc = kpool.tile([P, FT, P], MM_DT, tag="kc")
                eng.dma_start(out=kc, in_=kbs.ap()[t])
                kcs[t] = kc

            qT_sb = singles.tile([P, FT, NL], MM_DT)

            def _qt_dma(ft, eng):
                eng.dma_start(out=qT_sb[:, ft, :], in_=qT.ap()[:, ft, :])

            _kc_dma(0, nc.sync)
            _qt_dma(0, nc.scalar)
            _qt_dma(2, nc.gpsimd)
            _qt_dma(1, nc.sync)
            _kc_dma(1, nc.scalar)
            _qt_dma(5, nc.gpsimd)
            _qt_dma(3, nc.sync)
            _qt_dma(4, nc.scalar)
            _kc_dma(5, nc.gpsimd)
            _qt_dma(6, nc.sync)
            _qt_dma(7, nc.scalar)
            _kc_dma(7, nc.gpsimd)
            _kc_dma(2, nc.sync)
            _kc_dma(3, nc.scalar)
            _kc_dma(4, nc.sync)
            ones_sb = singles.tile([P, MT], MM_DT)
            nc.scalar.dma_start(out=ones_sb, in_=ones.ap())
            _kc_dma(6, nc.sync)
            # first v chunks + projT prefetched on gpsimd (needed mid-kernel)
            vc_pre = []
            for vh in range(2):
                vc = vpool.tile([P, MT // 2, P], MM_DT, tag="vc")
                nc.gpsimd.dma_start(out=vc, in_=vbk.ap()[0, vh])
                vc_pre.append(vc)
            projT_sb = singles.tile([P, FT, F], MM_DT)
            nc.gpsimd.dma_start(
                out=projT_sb,
                in_=projT.ap().rearrange("(t p) f -> p t f", p=P),
            )

            # ---- scores + exp:  pT[m, n] = exp(SCALE * sum_f kTr[f, m] qT[f, n])
            # key tile t only against the first _lw(t) query columns.
            pts = []
            for t in range(MT):
                L = _lw(t)
                ta = t + LOOKAHEAD
                if ta < MT:
                    _kc_dma(ta, nc.sync if ta % 2 == 0 else nc.scalar)
                kc = kcs.pop(t)
                ps = spsum.tile([P, NL], F32, tag="sps")
                for ft in range(FT):
                    nc.tensor.matmul(
                        ps[:, :L],
                        kc[:, ft, :],
                        qT_sb[:, ft, :L],
                        start=(ft == 0),
                        stop=(ft == FT - 1),
                    )
                pt = singles.tile([P, NL], MM_DT, tag=f"pt{t}")
                nc.scalar.activation(
                    out=pt[:, :L],
                    in_=ps[:, :L],
                    func=mybir.ActivationFunctionType.Exp,
                    scale=SCALE,
                )
                if t % 8 == 7:
                    # diagonal block of the slot owning columns [L-128, L)
                    nc.gpsimd.affine_select(
                        out=pt[:, L - P : L],
                        in_=pt[:, L - P : L],
                        pattern=[[1, P]],
                        compare_op=mybir.AluOpType.is_ge,
                        fill=0.0,
                        base=0,
                        channel_multiplier=-1,
                    )
                pts.append(pt)

            # ---- row sums (junk tiles excluded via per-core ones data)
            rps = rpsum.tile([1, NL], F32)
            for t in range(MT):
                nc.tensor.matmul(
                    rps[:, : _lw(t)],
                    ones_sb[:, t : t + 1],
                    pts[t][:, : _lw(t)],
                    start=(t == 0),
                    stop=(t == MT - 1),
                )
            rs_sb = singles.tile([1, NL], F32)
            nc.vector.tensor_copy(out=rs_sb, in_=rps)
            nc.sync.dma_start(out=rs_o.ap(), in_=rs_sb)

            # ---- z.T[f, n] = sum_m v[m, f] * pT[m, n]  (unnormalized)
            # two-stage projection: after z0..z3 exist, accumulate their
            # contribution into SBUF partials so only ft=4..7 remain at the end.
            z_tiles = []
            partials = {}
            for ft in range(FT):
                for vh in range(2):  # half-chunks of 16 key tiles
                    if ft == 0:
                        vc = vc_pre[vh]
                    else:
                        vc = vpool.tile([P, MT // 2, P], MM_DT, tag="vc")
                        if vh == 0:
                            eng = nc.gpsimd
                        else:
                            eng = nc.sync if ft % 2 == 0 else nc.scalar
                        eng.dma_start(out=vc, in_=vbk.ap()[ft, vh])
                    if vh == 0:
                        zps = zpsum.tile([P, NL], F32, tag="zps")
                    for mi in range(MT // 2):
                        t = vh * 16 + mi
                        L = _lw(t)
                        nc.tensor.matmul(
                            zps[:, :L],
                            vc[:, mi, :],
                            pts[t][:, :L],
                            start=(t == 0),
                            stop=(t == MT - 1),
                        )
                zt = singles.tile([P, NL], MM_DT, tag=f"z{ft}")
                nc.vector.tensor_copy(out=zt, in_=zps)
                z_tiles.append(zt)
                if ft == 3:
                    for nt in range(NT2):
                        for oc in range(2):
                            os_ = slice(oc * 512, (oc + 1) * 512)
                            ops = opsum.tile([P, 512], F32, tag="ops")
                            for fi in range(4):
                                nc.tensor.matmul(
                                    ops,
                                    z_tiles[fi][:, nt * P : (nt + 1) * P],
                                    projT_sb[:, fi, os_],
                                    start=(fi == 0),
                                    stop=(fi == 3),
                                )
                            part = singles.tile([P, 512], F32, tag=f"pp{nt}_{oc}")
                            nc.vector.tensor_copy(out=part, in_=ops)
                            partials[(nt, oc)] = part

            # ---- out[n, o] = z.T @ projT  (normalization + bias on host)
            for nt in range(NT2):
                for oc in range(2):
                    os_ = slice(oc * 512, (oc + 1) * 512)
                    ops = opsum.tile([P, 512], F32, tag="ops")
                    for fi in range(4, FT):
                        nc.tensor.matmul(
                            ops,
                            z_tiles[fi][:, nt * P : (nt + 1) * P],
                            projT_sb[:, fi, os_],
                            start=(fi == 4),
                            stop=(fi == FT - 1),
                        )
                    osb = opool.tile([P, 512], F32, tag="osb")
                    nc.vector.tensor_add(out=osb, in0=ops, in1=partials[(nt, oc)])
                    for half, eng in ((0, nc.scalar), (1, nc.sync)):
                        hs = slice(oc * 512 + half * 256, oc * 512 + half * 256 + 256)
                        eng.dma_start(
                            out=out_o.ap()[nt * P : (nt + 1) * P, hs],
                            in_=osb[:, half * 256 : half * 256 + 256],
                        )
    nc.finalize()
    return nc


def _get_programs():
    if "qkv" not in _CACHE:
        _CACHE["qkv"] = _build_qkv()
        _CACHE["attn"] = _build_attn()
    return _CACHE["qkv"], _CACHE["attn"]


def _c(a):
    return np.ascontiguousarray(a, dtype=np.float32)


def _b(a):
    return np.ascontiguousarray(np.asarray(a, dtype=np.float32).astype(ml_dtypes.bfloat16))


def _blocks_for_core(c):
    return [31 - c, 23 - c, 15 - c, 7 - c]


def kernel(x, wq_w, wq_b, wk_w, wk_b, wv_w, wv_b, proj_w, proj_b):
    x = np.asarray(x, dtype=np.float32)
    nc_qkv, nc_attn = _get_programs()

    # ---- launch A: QKV projection, sequence-sharded
    xT = np.asarray(x, dtype=np.float32).T        # [D, N]
    wqb = _b(np.asarray(wq_w).T.reshape(DT, P, FT, P).transpose(2, 1, 0, 3))
    wkb = _b(np.asarray(wk_w).T.reshape(DT, P, FT, P).transpose(2, 1, 0, 3))
    wvb = _b(np.asarray(wv_w).T.reshape(DT, P, 2, 512).transpose(2, 1, 0, 3))
    bq_pb = _c(np.asarray(wq_b).reshape(FT, P).T)   # [P, FT]
    bk_pb = _c(np.asarray(wk_b).reshape(FT, P).T)
    bvB = _c(np.broadcast_to(np.asarray(wv_b), (P, F)))
    in_a = []
    for c in range(C):
        xT_blk = _b(
            xT[:, c * NL : (c + 1) * NL].reshape(DT, P, NL).transpose(1, 0, 2)
        )
        in_a.append(
            {
                "xT": xT_blk,
                "wqb": wqb,
                "wkb": wkb,
                "wvb": wvb,
                "bq": bq_pb,
                "bk": bk_pb,
                "bvB": bvB,
            }
        )
    res_a = run_bass_kernel_spmd(nc_qkv, in_a, core_ids=list(range(C)))
    LAST_EXEC_NS[0] = res_a.exec_time_ns
    LAST_RESULTS[0] = res_a

    qT_full = np.concatenate([res_a.results[c]["qT_o"] for c in range(C)], axis=1)
    kT_full = np.concatenate([res_a.results[c]["kT_o"] for c in range(C)], axis=1)
    v_full = np.concatenate([res_a.results[c]["v_o"] for c in range(C)], axis=0)

    # ---- launch B: attention + projection, causally balanced
    projT = _b(np.asarray(proj_w).T)              # [F, F]
    in_b = []
    for c in range(C):
        qsel = np.concatenate(
            [qT_full[:, b * P : (b + 1) * P] for b in _blocks_for_core(c)], axis=1
        )  # [F, NL]
        qT_blk = np.ascontiguousarray(
            qsel.reshape(FT, P, NL).transpose(1, 0, 2)
        )
        # rotate keys/values by c tiles (junk zeros in tiles t < c)
        kTr = np.zeros((F, N), dtype=ml_dtypes.bfloat16)
        kTr[:, c * P :] = kT_full[:, : N - c * P]
        kbs = np.ascontiguousarray(
            kTr.reshape(FT, P, MT, P).transpose(2, 1, 0, 3)
        )
        vr = np.zeros((N, F), dtype=ml_dtypes.bfloat16)
        vr[c * P :, :] = v_full[: N - c * P]
        vbk = np.ascontiguousarray(
            vr.reshape(2, MT // 2, P, FT, P).transpose(3, 0, 2, 1, 4)
        )
        ones_pb = np.zeros((P, MT), dtype=ml_dtypes.bfloat16)
        ones_pb[:, c:] = 1.0
        in_b.append(
            {
                "qT": qT_blk,
                "kbs": kbs,
                "vbk": vbk,
                "ones": ones_pb,
                "projT": projT,
            }
        )
    res_b = run_bass_kernel_spmd(nc_attn, in_b, core_ids=list(range(C)))
    LAST_EXEC_NS[1] = res_b.exec_time_ns
    LAST_RESULTS[1] = res_b

    # ---- host: unshuffle rows, normalize, add bias (linear => exact)
    pb = np.asarray(proj_b, dtype=np.float32)
    out = np.empty((N, F), dtype=np.float32)
    for c in range(C):
        o_c = res_b.results[c]["out_o"]          # [NL, F] unnormalized
        r_c = res_b.results[c]["rs_o"][0]        # [NL]
        for j, blk in enumerate(_blocks_for_core(c)):
            rows = o_c[j * P : (j + 1) * P] / r_c[j * P : (j + 1) * P, None]
            out[blk * P : (blk + 1) * P] = rows + pb
    return out


# revision 30
# speedup vs baseline: 1.0189x; 1.0189x over previous
"""Causal single-head attention (N=4096, D=F=1024) on 8 TRN2 NeuronCores.

Causally load-balanced sequence sharding: core c owns the four 128-row query
blocks {31-c, 23-c, 15-c, 7-c}. Keys/values are rotated by c tiles (junk
zeros in tiles t<c) so each core runs one uniform SPMD program in which key
tile t is matmul'd against a compile-time prefix of the query columns
(512/384/256/128 wide for t in [0,8)/[8,16)/[16,24)/[24,32)) - the
shrinking prefix implements the causal structure at tile granularity and
cuts score/AV matmul work to 62.5% of the full rectangle. Diagonal blocks
(t = 7,15,23,31) get an on-chip triangular affine_select on their last 128
columns. Softmax normalization + output bias are applied host-side on the
unnormalized projected output (linear, so exact).

Two SPMD launches:
  A) QKV projection for the core's own contiguous 512 rows.
  B) scores+exp / rowsum / AV / output projection on the shuffled blocks.
Matmul operands bf16 (f32 PSUM accumulation); host pre-blocks all tensors
so every DMA is ~128 descriptors of >=2KB contiguous per partition.
"""

import sys

try:
    import concourse.bass as bass
except ImportError:  # pragma: no cover
    sys.path.insert(0, "/opt/trn_rl_repo")
    import concourse.bass as bass

import ml_dtypes
import numpy as np

import concourse.mybir as mybir
import concourse.tile as tile
from concourse import bacc
from concourse.bass_utils import run_bass_kernel_spmd

N, D, F = 4096, 1024, 1024
C = 8              # cores
NL = N // C        # 512 query rows per core
P = 128
SCALE = 1.0 / float(np.sqrt(np.float32(F)))

F32 = mybir.dt.float32
MM_DT = mybir.dt.bfloat16  # matmul operand dtype (PSUM accumulation stays f32)

DT = D // P        # 8 contraction tiles
FT = F // P        # 8 f tiles
MT = N // P        # 32 key tiles
NT2 = NL // P      # 4 query-row tiles per core

WARMUP_A = 12
WARMUP_B = 12

# column width of key tile t (prefix of the query columns)
def _lw(t):
    return 512 - 128 * (t // 8)


# Filled with [launchA_ns, launchB_ns] when BASS_TRACE=1 profiling is active.
LAST_EXEC_NS = [None, None]
LAST_RESULTS = [None, None]

_CACHE = {}


def _build_qkv():
    nc = bacc.Bacc(None, target_bir_lowering=False)
    xT = nc.dram_tensor("xT", [P, DT, NL], MM_DT, kind="ExternalInput")
    wqb = nc.dram_tensor("wqb", [FT, P, DT, P], MM_DT, kind="ExternalInput")
    wkb = nc.dram_tensor("wkb", [FT, P, DT, P], MM_DT, kind="ExternalInput")
    wvb = nc.dram_tensor("wvb", [2, P, DT, 512], MM_DT, kind="ExternalInput")
    bq = nc.dram_tensor("bq", [P, FT], F32, kind="ExternalInput")
    bk = nc.dram_tensor("bk", [P, FT], F32, kind="ExternalInput")
    bvB = nc.dram_tensor("bvB", [P, F], F32, kind="ExternalInput")
    qT_o = nc.dram_tensor("qT_o", [F, NL], MM_DT, kind="ExternalOutput")
    kT_o = nc.dram_tensor("kT_o", [F, NL], MM_DT, kind="ExternalOutput")
    v_o = nc.dram_tensor("v_o", [NL, F], MM_DT, kind="ExternalOutput")

    with tile.TileContext(nc) as tc:
        with (
            tc.tile_pool(name="singles", bufs=1) as singles,
            tc.tile_pool(name="weights", bufs=8) as weights,
            tc.tile_pool(name="osb", bufs=6) as opool,
            tc.tile_pool(name="psum", bufs=6, space="PSUM") as psum,
        ):
            warm = singles.tile([P, NL], MM_DT)
            nc.vector.memset(warm, 0.0)
            wps = psum.tile([P, NL], F32, tag="ps")
            for wi in range(WARMUP_A):
                nc.tensor.matmul(
                    wps,
                    warm[:, :P],
                    warm,
                    start=(wi == 0),
                    stop=(wi == WARMUP_A - 1),
                )
            # first two weight chunks prefetched ahead so MM0 starts early;
            # xT quarters fan out across all four DMA-capable queues
            wc0 = weights.tile([P, DT, P], MM_DT, tag="wc")
            nc.sync.dma_start(out=wc0, in_=wqb.ap()[0])
            wc1 = weights.tile([P, DT, P], MM_DT, tag="wc")
            nc.scalar.dma_start(out=wc1, in_=wqb.ap()[1])
            xT_sb = singles.tile([P, DT, NL], MM_DT)
            for qi, eng in (
                (0, nc.sync),
                (1, nc.scalar),
                (2, nc.gpsimd),
                (3, nc.gpsimd),
            ):
                sl = slice(qi * (DT // 4), (qi + 1) * (DT // 4))
                eng.dma_start(out=xT_sb[:, sl, :], in_=xT.ap()[:, sl, :])
            bq_sb = singles.tile([P, FT], F32)
            nc.gpsimd.dma_start(out=bq_sb, in_=bq.ap())
            bk_sb = singles.tile([P, FT], F32)
            nc.gpsimd.dma_start(out=bk_sb, in_=bk.ap())
            bvB_sb = singles.tile([P, F], F32)
            nc.gpsimd.dma_start(out=bvB_sb, in_=bvB.ap())

            # q.T / k.T : out[f_tile, n] = sum_d wT[d, f] * xT[d, n]
            for wi, (w_t, b_sb, out_t) in enumerate(
                ((wqb, bq_sb, qT_o), (wkb, bk_sb, kT_o))
            ):
                for ft in range(FT):
                    idx = wi * FT + ft
                    if idx == 0:
                        wc = wc0
                    elif idx == 1:
                        wc = wc1
                    else:
                        wc = weights.tile([P, DT, P], MM_DT, tag="wc")
                        weng = nc.sync if idx % 2 == 0 else nc.scalar
                        weng.dma_start(out=wc, in_=w_t.ap()[ft])
                    ps = psum.tile([P, NL], F32, tag="ps")
                    for dt_i in range(DT):
                        nc.tensor.matmul(
                            ps,
                            wc[:, dt_i, :],
                            xT_sb[:, dt_i, :],
                            start=(dt_i == 0),
                            stop=(dt_i == DT - 1),
                        )
                    osb = opool.tile([P, NL], MM_DT, tag="osb")
                    nc.vector.tensor_scalar_add(
                        out=osb, in0=ps, scalar1=b_sb[:, ft : ft + 1]
                    )
                    oeng = nc.scalar if idx % 2 == 0 else nc.sync
                    oeng.dma_start(
                        out=out_t.ap()[ft * P : (ft + 1) * P, :], in_=osb
                    )

            # v : out[m_tile, f] = sum_d xT[d, m] * wvT[d, f]
            for fc in range(2):
                fs = slice(fc * 512, (fc + 1) * 512)
                wvc = weights.tile([P, DT, 512], MM_DT, tag="wvc")
                nc.gpsimd.dma_start(out=wvc, in_=wvb.ap()[fc])
                for mi in range(NT2):
                    ps = psum.tile([P, 512], F32, tag="ps")
                    for dt_i in range(DT):
                        nc.tensor.matmul(
                            ps,
                            xT_sb[:, dt_i, mi * P : (mi + 1) * P],
                            wvc[:, dt_i, :],
                            start=(dt_i == 0),
                            stop=(dt_i == DT - 1),
                        )
                    vsb = opool.tile([P, 512], MM_DT, tag="osb")
                    nc.vector.tensor_add(out=vsb, in0=ps, in1=bvB_sb[:, fs])
                    veng = nc.scalar if mi % 2 == 0 else nc.sync
                    veng.dma_start(
                        out=v_o.ap()[mi * P : (mi + 1) * P, fs], in_=vsb
                    )
    nc.finalize()
    return nc


def _build_attn():
    nc = bacc.Bacc(None, target_bir_lowering=False)
    qT = nc.dram_tensor("qT", [P, FT, NL], MM_DT, kind="ExternalInput")
    kbs = nc.dram_tensor("kbs", [MT, P, FT, P], MM_DT, kind="ExternalInput")
    vbk = nc.dram_tensor("vbk", [FT, 2, P, MT // 2, P], MM_DT, kind="ExternalInput")
    ones = nc.dram_tensor("ones", [P, MT], MM_DT, kind="ExternalInput")
    projT = nc.dram_tensor("projT", [F, F], MM_DT, kind="ExternalInput")
    out_o = nc.dram_tensor("out_o", [NL, F], MM_DT, kind="ExternalOutput")
    rs_o = nc.dram_tensor("rs_o", [1, NL], F32, kind="ExternalOutput")

    with tile.TileContext(nc) as tc:
        with (
            tc.tile_pool(name="singles", bufs=1) as singles,
            tc.tile_pool(name="kc", bufs=10) as kpool,
            tc.tile_pool(name="vc", bufs=6) as vpool,
            tc.tile_pool(name="osb", bufs=3) as opool,
            tc.tile_pool(name="sps", bufs=3, space="PSUM") as spsum,
            tc.tile_pool(name="zps", bufs=2, space="PSUM") as zpsum,
            tc.tile_pool(name="ops", bufs=3, space="PSUM") as opsum,
        ):
            warm = singles.tile([P, NL], MM_DT)
            nc.vector.memset(warm, 0.0)
            wps = spsum.tile([P, NL], F32, tag="sps")
            for wi in range(WARMUP_B):
                nc.tensor.matmul(
                    wps,
                    warm[:, :P],
                    warm,
                    start=(wi == 0),
                    stop=(wi == WARMUP_B - 1),
                )
            # interleaved ramp: qT chunks + first 8 key tiles spread over all
            # three DMA queues so the ~1.5MB critical set lands fastest.
            LOOKAHEAD = 8
            kcs = {}

            def _kc_dma(t, eng):
                kc = kpool.tile([P, FT, P], MM_DT, tag="kc")
                eng.dma_start(out=kc, in_=kbs.ap()[t])
                kcs[t] = kc

            qT_sb = singles.tile([P, FT, NL], MM_DT)

            def _qt_dma(ft, eng):
                eng.dma_start(out=qT_sb[:, ft, :], in_=qT.ap()[:, ft, :])

            _kc_dma(0, nc.sync)
            _qt_dma(0, nc.scalar)
            _qt_dma(2, nc.gpsimd)
            _qt_dma(1, nc.sync)
            _qt_dma(3, nc.scalar)
            _qt_dma(5, nc.gpsimd)
            _qt_dma(4, nc.sync)
            _qt_dma(6, nc.scalar)
            _kc_dma(5, nc.gpsimd)
            _qt_dma(7, nc.sync)
            _kc_dma(1, nc.scalar)
            _kc_dma(7, nc.gpsimd)
            _kc_dma(2, nc.sync)
            _kc_dma(3, nc.scalar)
            _kc_dma(4, nc.sync)
            ones_sb = singles.tile([P, MT], MM_DT)
            nc.scalar.dma_start(out=ones_sb, in_=ones.ap())
            _kc_dma(6, nc.sync)
            # first v chunks + projT prefetched on gpsimd (needed mid-kernel)
            vc_pre = []
            for vh in range(2):
                vc = vpool.tile([P, MT // 2, P], MM_DT, tag="vc")
                nc.gpsimd.dma_start(out=vc, in_=vbk.ap()[0, vh])
                vc_pre.append(vc)
            projT_sb = singles.tile([P, FT, F], MM_DT)
            nc.gpsimd.dma_start(
                out=projT_sb,
                in_=projT.ap().rearrange("(t p) f -> p t f", p=P),
            )

            # ---- scores + exp:  pT[m, n] = exp(SCALE * sum_f kTr[f, m] qT[f, n])
            # key tile t only against the first _lw(t) query columns.
            pts = []
            for t in range(MT):
                L = _lw(t)
                ta = t + LOOKAHEAD
                if ta < MT:
                    _kc_dma(ta, nc.sync if ta % 2 == 0 else nc.scalar)
                kc = kcs.pop(t)
                ps = spsum.tile([P, NL], F32, tag="sps")
                for ft in range(FT):
                    nc.tensor.matmul(
                        ps[:, :L],
                        kc[:, ft, :],
                        qT_sb[:, ft, :L],
                        start=(ft == 0),
                        stop=(ft == FT - 1),
                    )
                pt = singles.tile([P, NL], MM_DT, tag=f"pt{t}")
                nc.scalar.activation(
                    out=pt[:, :L],
                    in_=ps[:, :L],
                    func=mybir.ActivationFunctionType.Exp,
                    scale=SCALE,
                )
                if t % 8 == 7:
                    # diagonal block of the slot owning columns [L-128, L)
                    nc.gpsimd.affine_select(
                        out=pt[:, L - P : L],
                        in_=pt[:, L - P : L],
                        pattern=[[1, P]],
                        compare_op=mybir.AluOpType.is_ge,
                        fill=0.0,
                        base=0,
                        channel_multiplier=-1,
                    )
                pts.append(pt)

            # ---- row sums (junk tiles excluded via per-core ones data)
            rps = opsum.tile([P, NL], F32, tag="ops")
            for t in range(MT):
                nc.tensor.matmul(
                    rps[0:1, : _lw(t)],
                    ones_sb[:, t : t + 1],
                    pts[t][:, : _lw(t)],
                    start=(t == 0),
                    stop=(t == MT - 1),
                )
            rs_sb = singles.tile([1, NL], F32)
            nc.vector.tensor_copy(out=rs_sb, in_=rps[0:1, :])
            nc.sync.dma_start(out=rs_o.ap(), in_=rs_sb)

            # ---- z.T[f, n] = sum_m v[m, f] * pT[m, n]  (unnormalized)
            # two-stage projection: after z0..z3 exist, accumulate their
            # contribution into SBUF partials so only ft=4..7 remain at the end.
            z_tiles = []
            partials = {}
            for ft in range(FT):
                for vh in range(2):  # half-chunks of 16 key tiles
                    if ft == 0:
                        vc = vc_pre[vh]
                    else:
                        vc = vpool.tile([P, MT // 2, P], MM_DT, tag="vc")
                        if vh == 0:
                            eng = nc.gpsimd
                        else:
                            eng = nc.sync if ft % 2 == 0 else nc.scalar
                        eng.dma_start(out=vc, in_=vbk.ap()[ft, vh])
                    if vh == 0:
                        zps = zpsum.tile([P, NL], F32, tag="zps")
                    for mi in range(MT // 2):
                        t = vh * 16 + mi
                        L = _lw(t)
                        nc.tensor.matmul(
                            zps[:, :L],
                            vc[:, mi, :],
                            pts[t][:, :L],
                            start=(t == 0),
                            stop=(t == MT - 1),
                        )
                zt = singles.tile([P, NL], MM_DT, tag=f"z{ft}")
                nc.vector.tensor_copy(out=zt, in_=zps)
                z_tiles.append(zt)
                if ft == 3:
                    for nt in range(NT2):
                        for oc in range(2):
                            os_ = slice(oc * 512, (oc + 1) * 512)
                            ops = opsum.tile([P, 512], F32, tag="ops")
                            for fi in range(4):
                                nc.tensor.matmul(
                                    ops,
                                    z_tiles[fi][:, nt * P : (nt + 1) * P],
                                    projT_sb[:, fi, os_],
                                    start=(fi == 0),
                                    stop=(fi == 3),
                                )
                            part = singles.tile([P, 512], F32, tag=f"pp{nt}_{oc}")
                            nc.vector.tensor_copy(out=part, in_=ops)
                            partials[(nt, oc)] = part

            # ---- out[n, o] = z.T @ projT  (normalization + bias on host)
            for nt in range(NT2):
                for oc in range(2):
                    os_ = slice(oc * 512, (oc + 1) * 512)
                    ops = opsum.tile([P, 512], F32, tag="ops")
                    for fi in range(4, FT):
                        nc.tensor.matmul(
                            ops,
                            z_tiles[fi][:, nt * P : (nt + 1) * P],
                            projT_sb[:, fi, os_],
                            start=(fi == 4),
                            stop=(fi == FT - 1),
                        )
                    osb = opool.tile([P, 512], MM_DT, tag="osb")
                    nc.vector.tensor_add(out=osb, in0=ops, in1=partials[(nt, oc)])
                    for half, eng in ((0, nc.scalar), (1, nc.sync)):
                        hs = slice(oc * 512 + half * 256, oc * 512 + half * 256 + 256)
                        eng.dma_start(
                            out=out_o.ap()[nt * P : (nt + 1) * P, hs],
                            in_=osb[:, half * 256 : half * 256 + 256],
                        )
    nc.finalize()
    return nc


def _get_programs():
    if "qkv" not in _CACHE:
        _CACHE["qkv"] = _build_qkv()
        _CACHE["attn"] = _build_attn()
    return _CACHE["qkv"], _CACHE["attn"]


def _c(a):
    return np.ascontiguousarray(a, dtype=np.float32)


def _b(a):
    return np.ascontiguousarray(np.asarray(a, dtype=np.float32).astype(ml_dtypes.bfloat16))


def _blocks_for_core(c):
    return [31 - c, 23 - c, 15 - c, 7 - c]


def kernel(x, wq_w, wq_b, wk_w, wk_b, wv_w, wv_b, proj_w, proj_b):
    x = np.asarray(x, dtype=np.float32)
    nc_qkv, nc_attn = _get_programs()

    # ---- launch A: QKV projection, sequence-sharded
    xT = np.asarray(x, dtype=np.float32).T        # [D, N]
    wqb = _b(np.asarray(wq_w).T.reshape(DT, P, FT, P).transpose(2, 1, 0, 3))
    wkb = _b(np.asarray(wk_w).T.reshape(DT, P, FT, P).transpose(2, 1, 0, 3))
    wvb = _b(np.asarray(wv_w).T.reshape(DT, P, 2, 512).transpose(2, 1, 0, 3))
    bq_pb = _c(np.asarray(wq_b).reshape(FT, P).T)   # [P, FT]
    bk_pb = _c(np.asarray(wk_b).reshape(FT, P).T)
    bvB = _c(np.broadcast_to(np.asarray(wv_b), (P, F)))
    in_a = []
    for c in range(C):
        xT_blk = _b(
            xT[:, c * NL : (c + 1) * NL].reshape(DT, P, NL).transpose(1, 0, 2)
        )
        in_a.append(
            {
                "xT": xT_blk,
                "wqb": wqb,
                "wkb": wkb,
                "wvb": wvb,
                "bq": bq_pb,
                "bk": bk_pb,
                "bvB": bvB,
            }
        )
    res_a = run_bass_kernel_spmd(nc_qkv, in_a, core_ids=list(range(C)))
    LAST_EXEC_NS[0] = res_a.exec_time_ns
    LAST_RESULTS[0] = res_a

    qT_full = np.concatenate([res_a.results[c]["qT_o"] for c in range(C)], axis=1)
    kT_full = np.concatenate([res_a.results[c]["kT_o"] for c in range(C)], axis=1)
    v_full = np.concatenate([res_a.results[c]["v_o"] for c in range(C)], axis=0)

    # ---- launch B: attention + projection, causally balanced
    projT = _b(np.asarray(proj_w).T)              # [F, F]
    in_b = []
    for c in range(C):
        qsel = np.concatenate(
            [qT_full[:, b * P : (b + 1) * P] for b in _blocks_for_core(c)], axis=1
        )  # [F, NL]
        qT_blk = np.ascontiguousarray(
            qsel.reshape(FT, P, NL).transpose(1, 0, 2)
        )
        # rotate keys/values by c tiles (junk zeros in tiles t < c)
        kTr = np.zeros((F, N), dtype=ml_dtypes.bfloat16)
        kTr[:, c * P :] = kT_full[:, : N - c * P]
        kbs = np.ascontiguousarray(
            kTr.reshape(FT, P, MT, P).transpose(2, 1, 0, 3)
        )
        vr = np.zeros((N, F), dtype=ml_dtypes.bfloat16)
        vr[c * P :, :] = v_full[: N - c * P]
        vbk = np.ascontiguousarray(
            vr.reshape(2, MT // 2, P, FT, P).transpose(3, 0, 2, 1, 4)
        )
        ones_pb = np.zeros((P, MT), dtype=ml_dtypes.bfloat16)
        ones_pb[:, c:] = 1.0
        in_b.append(
            {
                "qT": qT_blk,
                "kbs": kbs,
                "vbk": vbk,
                "ones": ones_pb,
                "projT": projT,
            }
        )
    res_b = run_bass_kernel_spmd(nc_attn, in_b, core_ids=list(range(C)))
    LAST_EXEC_NS[1] = res_b.exec_time_ns
    LAST_RESULTS[1] = res_b

    # ---- host: unshuffle rows, normalize, add bias (linear => exact)
    pb = np.asarray(proj_b, dtype=np.float32)
    out = np.empty((N, F), dtype=np.float32)
    for c in range(C):
        o_c = np.asarray(res_b.results[c]["out_o"], dtype=np.float32)
        r_c = res_b.results[c]["rs_o"][0]        # [NL]
        for j, blk in enumerate(_blocks_for_core(c)):
            rows = o_c[j * P : (j + 1) * P] / r_c[j * P : (j + 1) * P, None]
            out[blk * P : (blk + 1) * P] = rows + pb
    return out


# revision 40
# speedup vs baseline: 1.0836x; 1.0635x over previous
"""Causal single-head attention (N=4096, D=F=1024) on 8 TRN2 NeuronCores.

Causally load-balanced sequence sharding: core c owns the four 128-row query
blocks {31-c, 23-c, 15-c, 7-c}. Keys/values are rotated by c tiles (junk
zeros in tiles t<c) so each core runs one uniform SPMD program in which key
tile t is matmul'd against a compile-time prefix of the query columns
(512/384/256/128 wide for t in [0,8)/[8,16)/[16,24)/[24,32)) - the
shrinking prefix implements the causal structure at tile granularity and
cuts score/AV matmul work to 62.5% of the full rectangle. Diagonal blocks
(t = 7,15,23,31) get an on-chip triangular affine_select on their last 128
columns. Softmax normalization + output bias are applied host-side on the
unnormalized projected output (linear, so exact).

Two SPMD launches:
  A) QKV projection for the core's own contiguous 512 rows.
  B) scores+exp / rowsum / AV / output projection on the shuffled blocks.
Matmul operands bf16 (f32 PSUM accumulation); host pre-blocks all tensors
so every DMA is ~128 descriptors of >=2KB contiguous per partition.
"""

import sys

try:
    import concourse.bass as bass
except ImportError:  # pragma: no cover
    sys.path.insert(0, "/opt/trn_rl_repo")
    import concourse.bass as bass

import ml_dtypes
import numpy as np

import concourse.mybir as mybir
import concourse.tile as tile
from concourse import bacc
from concourse.bass_utils import run_bass_kernel_spmd

N, D, F = 4096, 1024, 1024
C = 8              # cores
NL = N // C        # 512 query rows per core
P = 128
SCALE = 1.0 / float(np.sqrt(np.float32(F)))

F32 = mybir.dt.float32
MM_DT = mybir.dt.bfloat16  # matmul operand dtype (PSUM accumulation stays f32)
QK_DT = mybir.dt.float8e3  # q/k score operands: e3m4, 4-bit mantissa

DT = D // P        # 8 contraction tiles
FT = F // P        # 8 f tiles
MT = N // P        # 32 key tiles
NT2 = NL // P      # 4 query-row tiles per core

WARMUP_A = 12
WARMUP_B = 12

# column width of key tile t (prefix of the query columns, 64-row blocks)
def _lw(t):
    return 512 - 64 * (t // 4)


# Filled with [launchA_ns, launchB_ns] when BASS_TRACE=1 profiling is active.
LAST_EXEC_NS = [None, None]
LAST_RESULTS = [None, None]

_CACHE = {}


def _build_qkv():
    nc = bacc.Bacc(None, target_bir_lowering=False)
    xT = nc.dram_tensor("xT", [P, DT, NL], MM_DT, kind="ExternalInput")
    wqb = nc.dram_tensor("wqb", [FT, P, DT, P], MM_DT, kind="ExternalInput")
    wkb = nc.dram_tensor("wkb", [FT, P, DT, P], MM_DT, kind="ExternalInput")
    wvb = nc.dram_tensor("wvb", [2, P, DT, 512], MM_DT, kind="ExternalInput")
    bq = nc.dram_tensor("bq", [P, FT], F32, kind="ExternalInput")
    bk = nc.dram_tensor("bk", [P, FT], F32, kind="ExternalInput")
    bvB = nc.dram_tensor("bvB", [P, F], F32, kind="ExternalInput")
    qT_o = nc.dram_tensor("qT_o", [F, NL], QK_DT, kind="ExternalOutput")
    kT_o = nc.dram_tensor("kT_o", [F, NL], QK_DT, kind="ExternalOutput")
    v_o = nc.dram_tensor("v_o", [NL, F], MM_DT, kind="ExternalOutput")

    with tile.TileContext(nc) as tc:
        with (
            tc.tile_pool(name="singles", bufs=1) as singles,
            tc.tile_pool(name="weights", bufs=8) as weights,
            tc.tile_pool(name="osb", bufs=6) as opool,
            tc.tile_pool(name="psum", bufs=6, space="PSUM") as psum,
        ):
            warm = singles.tile([P, NL], MM_DT)
            nc.vector.memset(warm, 0.0)
            wps = psum.tile([P, NL], F32, tag="ps")
            for wi in range(WARMUP_A):
                nc.tensor.matmul(
                    wps,
                    warm[:, :P],
                    warm,
                    start=(wi == 0),
                    stop=(wi == WARMUP_A - 1),
                )
            # first two weight chunks prefetched ahead so MM0 starts early;
            # xT quarters fan out across all four DMA-capable queues
            wc0 = weights.tile([P, DT, P], MM_DT, tag="wc")
            nc.sync.dma_start(out=wc0, in_=wqb.ap()[0])
            wc1 = weights.tile([P, DT, P], MM_DT, tag="wc")
            nc.scalar.dma_start(out=wc1, in_=wqb.ap()[1])
            xT_sb = singles.tile([P, DT, NL], MM_DT)
            for qi, eng in (
                (0, nc.sync),
                (1, nc.scalar),
                (2, nc.gpsimd),
                (3, nc.gpsimd),
            ):
                sl = slice(qi * (DT // 4), (qi + 1) * (DT // 4))
                eng.dma_start(out=xT_sb[:, sl, :], in_=xT.ap()[:, sl, :])
            bq_sb = singles.tile([P, FT], F32)
            nc.gpsimd.dma_start(out=bq_sb, in_=bq.ap())
            bk_sb = singles.tile([P, FT], F32)
            nc.gpsimd.dma_start(out=bk_sb, in_=bk.ap())
            bvB_sb = singles.tile([P, F], F32)
            nc.gpsimd.dma_start(out=bvB_sb, in_=bvB.ap())

            # q.T / k.T : out[f_tile, n] = sum_d wT[d, f] * xT[d, n]
            for wi, (w_t, b_sb, out_t) in enumerate(
                ((wqb, bq_sb, qT_o), (wkb, bk_sb, kT_o))
            ):
                for ft in range(FT):
                    idx = wi * FT + ft
                    if idx == 0:
                        wc = wc0
                    elif idx == 1:
                        wc = wc1
                    else:
                        wc = weights.tile([P, DT, P], MM_DT, tag="wc")
                        weng = nc.sync if idx % 2 == 0 else nc.scalar
                        weng.dma_start(out=wc, in_=w_t.ap()[ft])
                    ps = psum.tile([P, NL], F32, tag="ps")
                    for dt_i in range(DT):
                        nc.tensor.matmul(
                            ps,
                            wc[:, dt_i, :],
                            xT_sb[:, dt_i, :],
                            start=(dt_i == 0),
                            stop=(dt_i == DT - 1),
                        )
                    osb = opool.tile([P, NL], QK_DT, tag="osb")
                    nc.vector.tensor_scalar_add(
                        out=osb, in0=ps, scalar1=b_sb[:, ft : ft + 1]
                    )
                    oeng = nc.scalar if idx % 2 == 0 else nc.sync
                    oeng.dma_start(
                        out=out_t.ap()[ft * P : (ft + 1) * P, :], in_=osb
                    )

            # v : out[m_tile, f] = sum_d xT[d, m] * wvT[d, f]
            for fc in range(2):
                fs = slice(fc * 512, (fc + 1) * 512)
                wvc = weights.tile([P, DT, 512], MM_DT, tag="wvc")
                nc.gpsimd.dma_start(out=wvc, in_=wvb.ap()[fc])
                for mi in range(NT2):
                    ps = psum.tile([P, 512], F32, tag="ps")
                    for dt_i in range(DT):
                        nc.tensor.matmul(
                            ps,
                            xT_sb[:, dt_i, mi * P : (mi + 1) * P],
                            wvc[:, dt_i, :],
                            start=(dt_i == 0),
                            stop=(dt_i == DT - 1),
                        )
                    vsb = opool.tile([P, 512], MM_DT, tag="osb")
                    nc.vector.tensor_add(out=vsb, in0=ps, in1=bvB_sb[:, fs])
                    veng = nc.scalar if mi % 2 == 0 else nc.sync
                    veng.dma_start(
                        out=v_o.ap()[mi * P : (mi + 1) * P, fs], in_=vsb
                    )
    nc.finalize()
    return nc


def _build_attn():
    nc = bacc.Bacc(None, target_bir_lowering=False)
    qT = nc.dram_tensor("qT", [P, FT, NL], QK_DT, kind="ExternalInput")
    kbs = nc.dram_tensor("kbs", [MT, P, FT, P], QK_DT, kind="ExternalInput")
    vbk = nc.dram_tensor("vbk", [FT, 2, P, MT // 2, P], MM_DT, kind="ExternalInput")
    ones = nc.dram_tensor("ones", [P, MT], MM_DT, kind="ExternalInput")
    projT = nc.dram_tensor("projT", [F, F], MM_DT, kind="ExternalInput")
    out_o = nc.dram_tensor("out_o", [NL, F], MM_DT, kind="ExternalOutput")
    rs_o = nc.dram_tensor("rs_o", [1, NL], F32, kind="ExternalOutput")

    with tile.TileContext(nc) as tc:
        with (
            tc.tile_pool(name="singles", bufs=1) as singles,
            tc.tile_pool(name="kc", bufs=10) as kpool,
            tc.tile_pool(name="vc", bufs=6) as vpool,
            tc.tile_pool(name="osb", bufs=3) as opool,
            tc.tile_pool(name="sps", bufs=3, space="PSUM") as spsum,
            tc.tile_pool(name="zps", bufs=2, space="PSUM") as zpsum,
            tc.tile_pool(name="ops", bufs=3, space="PSUM") as opsum,
        ):
            warm = singles.tile([P, NL], MM_DT)
            nc.vector.memset(warm, 0.0)
            wps = spsum.tile([P, NL], F32, tag="sps")
            for wi in range(WARMUP_B):
                nc.tensor.matmul(
                    wps,
                    warm[:, :P],
                    warm,
                    start=(wi == 0),
                    stop=(wi == WARMUP_B - 1),
                )
            # interleaved ramp: qT chunks + first 8 key tiles spread over all
            # three DMA queues so the ~1.5MB critical set lands fastest.
            LOOKAHEAD = 8
            kcs = {}

            def _kc_dma(t, eng):
                kc = kpool.tile([P, FT, P], QK_DT, tag="kc")
                eng.dma_start(out=kc, in_=kbs.ap()[t])
                kcs[t] = kc

            qT_sb = singles.tile([P, FT, NL], QK_DT)

            def _qt_dma(ft, eng):
                eng.dma_start(out=qT_sb[:, ft, :], in_=qT.ap()[:, ft, :])

            _kc_dma(0, nc.sync)
            _qt_dma(0, nc.scalar)
            _qt_dma(2, nc.gpsimd)
            _qt_dma(1, nc.sync)
            _qt_dma(3, nc.scalar)
            _qt_dma(5, nc.gpsimd)
            _qt_dma(4, nc.sync)
            _qt_dma(6, nc.scalar)
            _kc_dma(5, nc.gpsimd)
            _qt_dma(7, nc.sync)
            _kc_dma(1, nc.scalar)
            _kc_dma(7, nc.gpsimd)
            _kc_dma(2, nc.sync)
            _kc_dma(3, nc.scalar)
            _kc_dma(4, nc.sync)
            ones_sb = singles.tile([P, MT], MM_DT)
            nc.scalar.dma_start(out=ones_sb, in_=ones.ap())
            _kc_dma(6, nc.sync)
            # first v chunks + projT prefetched on gpsimd (needed mid-kernel)
            vc_pre = []
            for vh in range(2):
                vc = vpool.tile([P, MT // 2, P], MM_DT, tag="vc")
                nc.gpsimd.dma_start(out=vc, in_=vbk.ap()[0, vh])
                vc_pre.append(vc)
            projT_sb = singles.tile([P, FT, F], MM_DT)
            nc.gpsimd.dma_start(
                out=projT_sb,
                in_=projT.ap().rearrange("(t p) f -> p t f", p=P),
            )

            # ---- scores + exp:  pT[m, n] = exp(SCALE * sum_f kTr[f, m] qT[f, n])
            # key tile t only against the first _lw(t) query columns.
            pts = []
            for t in range(MT):
                L = _lw(t)
                ta = t + LOOKAHEAD
                if ta < MT:
                    _kc_dma(ta, nc.sync if ta % 2 == 0 else nc.scalar)
                kc = kcs.pop(t)
                ps = spsum.tile([P, NL], F32, tag="sps")
                for ft in range(FT):
                    nc.tensor.matmul(
                        ps[:, :L],
                        kc[:, ft, :],
                        qT_sb[:, ft, :L],
                        start=(ft == 0),
                        stop=(ft == FT - 1),
                    )
                pt = singles.tile([P, NL], MM_DT, tag=f"pt{t}")
                nc.scalar.activation(
                    out=pt[:, :L],
                    in_=ps[:, :L],
                    func=mybir.ActivationFunctionType.Exp,
                    scale=SCALE,
                )
                if t % 4 == 3:
                    # diagonal 128-key x 64-query block: keep key r <= 64 + n
                    nc.gpsimd.affine_select(
                        out=pt[:, L - 64 : L],
                        in_=pt[:, L - 64 : L],
                        pattern=[[1, 64]],
                        compare_op=mybir.AluOpType.is_ge,
                        fill=0.0,
                        base=64,
                        channel_multiplier=-1,
                    )
                pts.append(pt)

            # ---- row sums (junk tiles excluded via per-core ones data)
            rps = opsum.tile([P, NL], F32, tag="ops")
            for t in range(MT):
                nc.tensor.matmul(
                    rps[0:1, : _lw(t)],
                    ones_sb[:, t : t + 1],
                    pts[t][:, : _lw(t)],
                    start=(t == 0),
                    stop=(t == MT - 1),
                )
            rs_sb = singles.tile([1, NL], F32)
            nc.vector.tensor_copy(out=rs_sb, in_=rps[0:1, :])
            nc.sync.dma_start(out=rs_o.ap(), in_=rs_sb)

            # ---- z.T[f, n] = sum_m v[m, f] * pT[m, n]  (unnormalized)
            # two-stage projection: after z0..z3 exist, accumulate their
            # contribution into SBUF partials so only ft=4..7 remain at the end.
            z_tiles = []
            partials = {}
            for ft in range(FT):
                for vh in range(2):  # half-chunks of 16 key tiles
                    if ft == 0:
                        vc = vc_pre[vh]
                    else:
                        vc = vpool.tile([P, MT // 2, P], MM_DT, tag="vc")
                        if vh == 0:
                            eng = nc.gpsimd
                        else:
                            eng = nc.sync if ft % 2 == 0 else nc.scalar
                        eng.dma_start(out=vc, in_=vbk.ap()[ft, vh])
                    if vh == 0:
                        zps = zpsum.tile([P, NL], F32, tag="zps")
                    for mi in range(MT // 2):
                        t = vh * 16 + mi
                        L = _lw(t)
                        nc.tensor.matmul(
                            zps[:, :L],
                            vc[:, mi, :],
                            pts[t][:, :L],
                            start=(t == 0),
                            stop=(t == MT - 1),
                        )
                zt = singles.tile([P, NL], MM_DT, tag=f"z{ft}")
                nc.vector.tensor_copy(out=zt, in_=zps)
                z_tiles.append(zt)
                if ft == 3:
                    for nt in range(NT2):
                        for oc in range(2):
                            os_ = slice(oc * 512, (oc + 1) * 512)
                            ops = opsum.tile([P, 512], F32, tag="ops")
                            for fi in range(4):
                                nc.tensor.matmul(
                                    ops,
                                    z_tiles[fi][:, nt * P : (nt + 1) * P],
                                    projT_sb[:, fi, os_],
                                    start=(fi == 0),
                                    stop=(fi == 3),
                                )
                            part = singles.tile([P, 512], F32, tag=f"pp{nt}_{oc}")
                            nc.vector.tensor_copy(out=part, in_=ops)
                            partials[(nt, oc)] = part

            # ---- out[n, o] = z.T @ projT  (normalization + bias on host)
            for nt in range(NT2):
                for oc in range(2):
                    os_ = slice(oc * 512, (oc + 1) * 512)
                    ops = opsum.tile([P, 512], F32, tag="ops")
                    for fi in range(4, FT):
                        nc.tensor.matmul(
                            ops,
                            z_tiles[fi][:, nt * P : (nt + 1) * P],
                            projT_sb[:, fi, os_],
                            start=(fi == 4),
                            stop=(fi == FT - 1),
                        )
                    osb = opool.tile([P, 512], MM_DT, tag="osb")
                    nc.vector.tensor_add(out=osb, in0=ops, in1=partials[(nt, oc)])
                    for half, eng in ((0, nc.scalar), (1, nc.sync)):
                        hs = slice(oc * 512 + half * 256, oc * 512 + half * 256 + 256)
                        eng.dma_start(
                            out=out_o.ap()[nt * P : (nt + 1) * P, hs],
                            in_=osb[:, half * 256 : half * 256 + 256],
                        )
    nc.finalize()
    return nc


def _get_programs():
    if "qkv" not in _CACHE:
        _CACHE["qkv"] = _build_qkv()
        _CACHE["attn"] = _build_attn()
    return _CACHE["qkv"], _CACHE["attn"]


def _c(a):
    return np.ascontiguousarray(a, dtype=np.float32)


def _b(a):
    return np.ascontiguousarray(np.asarray(a, dtype=np.float32).astype(ml_dtypes.bfloat16))


def _blocks_for_core(c):
    # 64-row query blocks, col group g owns block 63 - 8g - c
    return [63 - 8 * g - c for g in range(8)]


def kernel(x, wq_w, wq_b, wk_w, wk_b, wv_w, wv_b, proj_w, proj_b):
    x = np.asarray(x, dtype=np.float32)
    nc_qkv, nc_attn = _get_programs()

    # ---- launch A: QKV projection, sequence-sharded
    xT = np.asarray(x, dtype=np.float32).T        # [D, N]
    wqb = _b(np.asarray(wq_w).T.reshape(DT, P, FT, P).transpose(2, 1, 0, 3))
    wkb = _b(np.asarray(wk_w).T.reshape(DT, P, FT, P).transpose(2, 1, 0, 3))
    wvb = _b(np.asarray(wv_w).T.reshape(DT, P, 2, 512).transpose(2, 1, 0, 3))
    bq_pb = _c(np.asarray(wq_b).reshape(FT, P).T)   # [P, FT]
    bk_pb = _c(np.asarray(wk_b).reshape(FT, P).T)
    bvB = _c(np.broadcast_to(np.asarray(wv_b), (P, F)))
    in_a = []
    for c in range(C):
        xT_blk = _b(
            xT[:, c * NL : (c + 1) * NL].reshape(DT, P, NL).transpose(1, 0, 2)
        )
        in_a.append(
            {
                "xT": xT_blk,
                "wqb": wqb,
                "wkb": wkb,
                "wvb": wvb,
                "bq": bq_pb,
                "bk": bk_pb,
                "bvB": bvB,
            }
        )
    res_a = run_bass_kernel_spmd(nc_qkv, in_a, core_ids=list(range(C)))
    LAST_EXEC_NS[0] = res_a.exec_time_ns
    LAST_RESULTS[0] = res_a

    qT_full = np.concatenate([res_a.results[c]["qT_o"] for c in range(C)], axis=1)
    kT_full = np.concatenate([res_a.results[c]["kT_o"] for c in range(C)], axis=1)
    v_full = np.concatenate([res_a.results[c]["v_o"] for c in range(C)], axis=0)

    # ---- launch B: attention + projection, causally balanced
    projT = _b(np.asarray(proj_w).T)              # [F, F]
    in_b = []
    for c in range(C):
        qsel = np.concatenate(
            [qT_full[:, b * 64 : (b + 1) * 64] for b in _blocks_for_core(c)], axis=1
        )  # [F, NL]
        qT_blk = np.ascontiguousarray(
            qsel.reshape(FT, P, NL).transpose(1, 0, 2)
        )
        # rotate keys/values by 64*c rows (junk zeros in rotated rows < 64c)
        sh = 64 * c
        kTr = np.zeros((F, N), dtype=ml_dtypes.float8_e3m4)
        kTr[:, sh:] = kT_full[:, : N - sh]
        kbs = np.ascontiguousarray(
            kTr.reshape(FT, P, MT, P).transpose(2, 1, 0, 3)
        )
        vr = np.zeros((N, F), dtype=ml_dtypes.bfloat16)
        vr[sh:, :] = v_full[: N - sh]
        vbk = np.ascontiguousarray(
            vr.reshape(2, MT // 2, P, FT, P).transpose(3, 0, 2, 1, 4)
        )
        # ones[r, t] gates key (128t + r); zero for junk rotated rows < 64c
        key_idx = np.arange(N).reshape(MT, P).T  # [P, MT]
        ones_pb = (key_idx >= sh).astype(ml_dtypes.bfloat16)
        in_b.append(
            {
                "qT": qT_blk,
                "kbs": kbs,
                "vbk": vbk,
                "ones": ones_pb,
                "projT": projT,
            }
        )
    res_b = run_bass_kernel_spmd(nc_attn, in_b, core_ids=list(range(C)))
    LAST_EXEC_NS[1] = res_b.exec_time_ns
    LAST_RESULTS[1] = res_b

    # ---- host: unshuffle rows, normalize, add bias (linear => exact)
    pb = np.asarray(proj_b, dtype=np.float32)
    out = np.empty((N, F), dtype=np.float32)
    for c in range(C):
        o_c = np.asarray(res_b.results[c]["out_o"], dtype=np.float32)
        r_c = res_b.results[c]["rs_o"][0]        # [NL]
        for j, blk in enumerate(_blocks_for_core(c)):
            rows = o_c[j * 64 : (j + 1) * 64] / r_c[j * 64 : (j + 1) * 64, None]
            out[blk * 64 : (blk + 1) * 64] = rows + pb
    return out
